# revision 21
# baseline (speedup 1.0000x reference)
"""Trainium2 Bass kernel for a dense transformer block (LN1 -> MHA -> LN2 -> MLP).

Sharding: 8 cores = (batch b in 0..3) x (sequence half in 0..1). Each core
computes the block output for its 1024 query tokens; K/V are computed for the
batch's full 2048 tokens on each core (replicated within the pair), so there
is zero cross-core communication.

Layout: on-chip activations are transposed ([feature, token]) so matmul
chains compose without transposes; the host transposes x per core and
transposes the per-core outputs back.

Dtypes: attention path bf16 (Q/K/V/probs), residuals fp32, MLP float32r
(full PE speed at N>=256, ~1e-4 matmul accuracy), LN stats via bf16 PE
ones-matmuls (rounding noise averages out across 1024 terms).
"""

import sys

if '/opt/trn_rl_repo' not in sys.path:
    sys.path.insert(0, '/opt/trn_rl_repo')

import numpy as np
import ml_dtypes

import concourse.tile as tile
import concourse.mybir as mybir
from concourse import bacc
from concourse.bass import ts
from concourse.bass_utils import run_bass_kernel_spmd

P = 128
F32 = mybir.dt.float32
F32R = mybir.dt.float32r
BF16 = mybir.dt.bfloat16
AF = mybir.ActivationFunctionType
EPS = 1e-6

B, S, D, H, MLP = 4, 2048, 1024, 16, 4096
N_CORES = 8


def _layernorm(nc, ones_h, eps_t, p_bc, p_tmp, p_st, ps_st, src_fn, n_dc, Tn, TBn,
               g_t, b_t, out_fn, dram_src=None):
    """LayerNorm along the feature (partition-chunk) direction.

    src_fn(dc) -> [P, Tn] fp32 AP of a resident tile, or None with dram_src
    set to a [Dm, Tn] fp32 dram AP to stream chunks (two passes over dram).
    out_fn(dc) -> [P, Tn] dest AP (any dtype).
    Feature sums via PE ones-matmuls on bf16 casts.
    """
    n_tb = Tn // TBn
    inv_d = 1.0 / (n_dc * P)
    if dram_src is None:
        mean_bc_full = p_bc.tile([P, Tn], F32, tag="ln_meanbc")
        rstd_bc_full = p_bc.tile([P, Tn], F32, tag="ln_rstdbc")
    for tb in range(n_tb):
        sl = ts(tb, TBn)
        ps_m = ps_st.tile([1, TBn], F32, tag="ps_stat")
        ps_s = ps_st.tile([1, TBn], F32, tag="ps_stat")
        for dc in range(n_dc):
            st, sp = (dc == 0), (dc == n_dc - 1)
            if dram_src is not None:
                xc = p_tmp.tile([P, TBn], F32, tag="ln_xc")
                nc.sync.dma_start(xc[:], dram_src[ts(dc, P), sl])
                src_sl = xc[:]
            else:
                src_sl = src_fn(dc)[:, sl]
            xb = p_tmp.tile([P, TBn], BF16, tag="ln_xb")
            nc.vector.tensor_copy(xb[:], src_sl)
            nc.tensor.matmul(ps_m[:], ones_h[:], xb[:], start=st, stop=sp)
            xsq = p_tmp.tile([P, TBn], BF16, tag="ln_xsq")
            nc.scalar.activation(xsq[:], src_sl, AF.Square)
            nc.tensor.matmul(ps_s[:], ones_h[:], xsq[:], start=st, stop=sp)
        mean = p_st.tile([1, TBn], F32)
        nc.vector.tensor_scalar_mul(mean[:], ps_m[:], inv_d)
        ex2 = p_st.tile([1, TBn], F32)
        nc.vector.tensor_scalar_mul(ex2[:], ps_s[:], inv_d)
        var = p_st.tile([1, TBn], F32)
        nc.vector.tensor_mul(var[:], mean[:], mean[:])
        nc.vector.tensor_sub(var[:], ex2[:], var[:])
        std = p_st.tile([1, TBn], F32)
        nc.scalar.activation(std[:], var[:], AF.Sqrt, bias=eps_t[:, :])
        rstd = p_st.tile([1, TBn], F32)
        nc.vector.reciprocal(rstd[:], std[:])
        if dram_src is None:
            nc.gpsimd.partition_broadcast(mean_bc_full[:, sl], mean[:])
            nc.gpsimd.partition_broadcast(rstd_bc_full[:, sl], rstd[:])
        else:
            # chunked apply: broadcast per token-block, re-stream source
            mean_bc = p_tmp.tile([P, TBn], F32, tag="ln_meanbc_c")
            rstd_bc = p_tmp.tile([P, TBn], F32, tag="ln_rstdbc_c")
            nc.gpsimd.partition_broadcast(mean_bc[:], mean[:])
            nc.gpsimd.partition_broadcast(rstd_bc[:], rstd[:])
            for dc in range(n_dc):
                t0 = p_tmp.tile([P, TBn], F32, tag="ln_xa")
                nc.sync.dma_start(t0[:], dram_src[ts(dc, P), sl])
                nc.vector.tensor_sub(t0[:], t0[:], mean_bc[:])
                nc.vector.tensor_mul(t0[:], t0[:], rstd_bc[:])
                nc.scalar.activation(out_fn(dc)[:, sl], t0[:], AF.Identity,
                                     bias=b_t[:, dc:dc + 1],
                                     scale=g_t[:, dc:dc + 1])
    if dram_src is None:
        for dc in range(n_dc):
            t0 = p_tmp.tile([P, Tn], F32, tag="ln_t0")
            nc.vector.tensor_sub(t0[:], src_fn(dc), mean_bc_full[:])
            nc.vector.tensor_mul(t0[:], t0[:], rstd_bc_full[:])
            nc.scalar.activation(out_fn(dc), t0[:], AF.Identity,
                                 bias=b_t[:, dc:dc + 1], scale=g_t[:, dc:dc + 1])


def build_bass(T, Q, Dm, Hh, Mlp, n_cores, dbg=False):
    dh = Dm // Hh
    assert dh == 64, "head packing assumes DH=64"
    n_dc = Dm // P
    n_tk = T // P
    TB = min(512, T)
    n_tb = T // TB
    QB = min(512, Q)
    n_qb = Q // QB
    QQ = min(512, Q)
    n_qq = Q // QQ
    n_mo = Mlp // P
    n_hp = Hh // 2

    nc = bacc.Bacc("TRN2", target_bir_lowering=False, debug=False,
                   enable_asserts=False, num_devices=n_cores)

    def din(name, shape, dt):
        return nc.dram_tensor(name, shape, dt, kind="ExternalInput").ap()

    xT_d = din("xT", (Dm, T), F32)
    xqT_d = din("xqT", (Dm, Q), F32)
    g1_d, be1_d = din("g1", (Dm,), F32), din("be1", (Dm,), F32)
    g2_d, be2_d = din("g2", (Dm,), F32), din("be2", (Dm,), F32)
    wq_d, wk_d = din("wq16", (Dm, Dm), BF16), din("wk16", (Dm, Dm), BF16)
    wv_d, wo_d = din("wv16", (Dm, Dm), BF16), din("wo16", (Dm, Dm), BF16)
    w1_d = din("w1r", (Dm, Mlp), F32R)
    w2_d = din("w2r", (Mlp, Dm), F32R)
    bq_d, bk_d = din("bq", (Dm,), F32), din("bk", (Dm,), F32)
    bv_d, bo_d = din("bv", (Dm,), F32), din("bo", (Dm,), F32)
    b1_d, b2_d = din("b1", (Mlp,), F32), din("b2", (Dm,), F32)
    ones_d = din("ones16", (P, 1), BF16)
    yT_d = nc.dram_tensor("yT", (Dm, Q), F32, kind="ExternalOutput").ap()
    dbg_d = {}
    if dbg:
        for nm, shape, dt in [("dXN", (Dm, T), BF16), ("dXNQ", (Dm, Q), BF16),
                              ("dKT", (Dm, T), BF16), ("dQT", (Dm, Q), BF16),
                              ("dVT", (T, Dm), BF16), ("dCT", (Dm, Q), BF16),
                              ("dh2", (Dm, Q), F32), ("drbc", (P, Q), F32),
                              ("dexp", (T, Q), BF16)]:
            dbg_d[nm] = nc.dram_tensor(nm, shape, dt, kind="ExternalOutput").ap()

    with tile.TileContext(nc) as tc:
        with tc.tile_pool(name="const", bufs=1) as constp:
            ones_h = constp.tile([P, 1], BF16)
            nc.sync.dma_start(ones_h[:], ones_d[:, :])
            eps_t = constp.tile([1, 1], F32)
            nc.vector.memset(eps_t[:], EPS)
            ones_f = constp.tile([P, P], BF16)
            nc.vector.memset(ones_f[:], 1.0)

            def vec_tile(src, n, nm):
                t = constp.tile([P, n], F32, tag=nm, name=nm)
                nc.sync.dma_start(t[:], src.rearrange("(c p) -> p c", p=P))
                return t

            g1_t, be1_t = vec_tile(g1_d, n_dc, "g1"), vec_tile(be1_d, n_dc, "be1")
            g2_t, be2_t = vec_tile(g2_d, n_dc, "g2"), vec_tile(be2_d, n_dc, "be2")
            bq_t, bk_t = vec_tile(bq_d, n_dc, "bq"), vec_tile(bk_d, n_dc, "bk")
            bo_t, b2_t = vec_tile(bo_d, n_dc, "bo"), vec_tile(b2_d, n_dc, "b2")
            b1_t = vec_tile(b1_d, n_mo, "b1")
            # bv broadcast along free dim (V is [token, d_out])
            bv_row = constp.tile([1, Dm], F32)
            nc.sync.dma_start(bv_row[:, :], bv_d[None, :])
            bv_bc = constp.tile([P, Dm], F32)
            nc.gpsimd.partition_broadcast(bv_bc[:], bv_row[:])

            with tc.tile_pool(name="p_h2", bufs=1) as p_h2:
                XQ = p_h2.tile([P, n_dc, Q], F32)  # x_q, becomes h2

                with tc.tile_pool(name="p_kv", bufs=1) as p_kv:
                    KT = p_kv.tile([P, n_dc, T], BF16)
                    VT = p_kv.tile([P, n_tk, Dm], BF16)
                    QT = p_kv.tile([P, n_dc, Q], BF16)

                    # ---------- Phase 1: LN1 + QKV ----------
                    with tc.tile_pool(name="p_act", bufs=1) as p_act, \
                         tc.tile_pool(name="p_str", bufs=4) as p_str, \
                         tc.tile_pool(name="p_tmp", bufs=2) as p_tmp, \
                         tc.tile_pool(name="p_st", bufs=1) as p_st, \
                         tc.tile_pool(name="ps_st", bufs=2, space="PSUM") as ps_st, \
                         tc.tile_pool(name="ps_mm", bufs=6, space="PSUM") as ps_mm:

                        XN = p_act.tile([P, n_dc, T], BF16)
                        _layernorm(nc, ones_h, eps_t, p_act, p_tmp, p_st, ps_st,
                                   None, n_dc, T, TB,
                                   g1_t, be1_t, lambda dc: XN[:, dc, :],
                                   dram_src=xT_d)
                        XNQ = p_act.tile([P, n_dc, Q], BF16)
                        _layernorm(nc, ones_h, eps_t, p_act, p_tmp, p_st, ps_st,
                                   None, n_dc, Q, QB,
                                   g1_t, be1_t, lambda dc: XNQ[:, dc, :],
                                   dram_src=xqT_d)

                        if dbg:
                            for dc in range(n_dc):
                                nc.sync.dma_start(dbg_d["dXN"][ts(dc, P), :], XN[:, dc, :])
                                nc.sync.dma_start(dbg_d["dXNQ"][ts(dc, P), :], XNQ[:, dc, :])
                        # K^T: lhsT = Wk chunk, rhs = XN
                        for mo in range(n_dc):
                            pss = [ps_mm.tile([P, TB], F32, tag="ps_mm", name="ps_mm")
                                   for _ in range(n_tb)]
                            for dc in range(n_dc):
                                wt = p_str.tile([P, P], BF16, tag="wkq")
                                nc.sync.dma_start(wt[:], wk_d[ts(dc, P), ts(mo, P)])
                                for tb in range(n_tb):
                                    nc.tensor.matmul(
                                        pss[tb][:], wt[:], XN[:, dc, ts(tb, TB)],
                                        start=(dc == 0), stop=(dc == n_dc - 1))
                            for tb in range(n_tb):
                                nc.scalar.activation(KT[:, mo, ts(tb, TB)],
                                                     pss[tb][:], AF.Identity,
                                                     bias=bk_t[:, mo:mo + 1])
                        # Q^T from XNQ
                        for mo in range(n_dc):
                            pss = [ps_mm.tile([P, QB], F32, tag="ps_mm", name="ps_mm")
                                   for _ in range(n_qb)]
                            for dc in range(n_dc):
                                wt = p_str.tile([P, P], BF16, tag="wkq")
                                nc.sync.dma_start(wt[:], wq_d[ts(dc, P), ts(mo, P)])
                                for qb in range(n_qb):
                                    nc.tensor.matmul(
                                        pss[qb][:], wt[:], XNQ[:, dc, ts(qb, QB)],
                                        start=(dc == 0), stop=(dc == n_dc - 1))
                            for qb in range(n_qb):
                                nc.scalar.activation(QT[:, mo, ts(qb, QB)],
                                                     pss[qb][:], AF.Identity,
                                                     bias=bq_t[:, mo:mo + 1])
                        # V: lhsT = XN chunk (tokens as M), rhs = Wv streamed
                        # per token-group (re-read n_tk/TG times)
                        NO = min(TB, Dm)
                        n_no = Dm // NO
                        TG = 4
                        for tg in range(0, n_tk, TG):
                            tos = range(tg, min(tg + TG, n_tk))
                            for no in range(n_no):
                                pss = [ps_mm.tile([P, NO], F32, tag="ps_mm",
                                                  name="ps_mm") for _ in tos]
                                for dc in range(n_dc):
                                    wvt = p_str.tile([P, NO], BF16, tag="wv")
                                    nc.sync.dma_start(
                                        wvt[:], wv_d[ts(dc, P), ts(no, NO)])
                                    for i, to in enumerate(tos):
                                        nc.tensor.matmul(
                                            pss[i][:], XN[:, dc, ts(to, P)],
                                            wvt[:],
                                            start=(dc == 0), stop=(dc == n_dc - 1))
                                for i, to in enumerate(tos):
                                    nc.vector.tensor_add(VT[:, to, ts(no, NO)],
                                                         pss[i][:],
                                                         bv_bc[:, ts(no, NO)])

                    if dbg:
                        for dc in range(n_dc):
                            nc.sync.dma_start(dbg_d["dKT"][ts(dc, P), :], KT[:, dc, :])
                            nc.sync.dma_start(dbg_d["dQT"][ts(dc, P), :], QT[:, dc, :])
                        for to in range(n_tk):
                            nc.sync.dma_start(dbg_d["dVT"][ts(to, P), :], VT[:, to, :])
                    # ---------- Phase 2: attention ----------
                    with tc.tile_pool(name="p_attn", bufs=1) as p_attn:
                        CT = p_attn.tile([P, n_dc, Q], BF16)
                        for dc in range(n_dc):
                            nc.sync.dma_start(XQ[:, dc, :], xqT_d[ts(dc, P), :])
                        with tc.tile_pool(name="p_exp", bufs=3) as p_exp, \
                             tc.tile_pool(name="p_rb", bufs=3) as p_rb, \
                             tc.tile_pool(name="ps_sc", bufs=2, space="PSUM") as ps_sc, \
                             tc.tile_pool(name="ps_ctx", bufs=2, space="PSUM") as ps_ctx, \
                             tc.tile_pool(name="ps_dn", bufs=2, space="PSUM") as ps_dn, \
                             tc.tile_pool(name="ps_rb", bufs=2, space="PSUM") as ps_rb:
                            for qq in range(n_qq):
                                qsl = ts(qq, QQ)
                                for hp in range(n_hp):
                                    exps = []
                                    for hi in range(2):
                                        r0 = hi * 64
                                        expT = p_exp.tile([P, n_tk, QQ], BF16,
                                                          tag="expT")
                                        for kc in range(n_tk):
                                            ps_s = ps_sc.tile([P, QQ], F32,
                                                              tag="ps_s")
                                            nc.tensor.matmul(
                                                ps_s[:],
                                                KT[r0:r0 + 64, hp, ts(kc, P)],
                                                QT[r0:r0 + 64, hp, qsl],
                                                start=True, stop=True)
                                            nc.scalar.activation(
                                                expT[:, kc, :], ps_s[:], AF.Exp,
                                                scale=0.125)
                                        exps.append(expT)
                                    rbcs = []
                                    for hi in range(2):
                                        ps_d = ps_dn.tile([P, QQ], F32, tag="ps_d",
                                                          name="ps_d")
                                        for kc in range(n_tk):
                                            nc.tensor.matmul(
                                                ps_d[:], ones_f[:],
                                                exps[hi][:, kc, :],
                                                start=(kc == 0),
                                                stop=(kc == n_tk - 1))
                                        rbc_h = p_rb.tile([P, QQ], F32, tag="rbc",
                                                          name="rbc")
                                        nc.vector.reciprocal(rbc_h[:], ps_d[:])
                                        rbcs.append(rbc_h)
                                    if dbg and hp == 0:
                                        nc.sync.dma_start(dbg_d["drbc"][0:64, qsl], rbcs[0][0:64, :])
                                        nc.sync.dma_start(dbg_d["drbc"][64:128, qsl], rbcs[1][64:128, :])
                                        for kc in range(n_tk):
                                            nc.sync.dma_start(
                                                dbg_d["dexp"][ts(kc, P), qsl],
                                                exps[0][:, kc, :])
                                    ps_c = ps_ctx.tile([P, QQ], F32, tag="ps_c")
                                    for hi in range(2):
                                        h = 2 * hp + hi
                                        for kc in range(n_tk):
                                            nc.tensor.matmul(
                                                ps_c[hi * 64:hi * 64 + 64, :],
                                                VT[:, kc, ts(h, 64)],
                                                exps[hi][:, kc, :],
                                                start=(kc == 0),
                                                stop=(kc == n_tk - 1),
                                                tile_position=(0, hi * 64))
                                    for hi in range(2):
                                        r0 = hi * 64
                                        nc.vector.tensor_mul(
                                            CT[r0:r0 + 64, hp, qsl],
                                            ps_c[r0:r0 + 64, :],
                                            rbcs[hi][r0:r0 + 64, :])

                        if dbg:
                            for dc in range(n_dc):
                                nc.sync.dma_start(dbg_d["dCT"][ts(dc, P), :], CT[:, dc, :])
                        # Wo + bias + residual into XQ
                        with tc.tile_pool(name="p_wos", bufs=4) as p_wos, \
                             tc.tile_pool(name="ps_wo", bufs=4, space="PSUM") as ps_wo:
                            for mo in range(n_dc):
                                pss = [ps_wo.tile([P, QB], F32, tag="ps_wo", name="ps_wo")
                                       for _ in range(n_qb)]
                                for dc in range(n_dc):
                                    wt = p_wos.tile([P, P], BF16, tag="wo")
                                    nc.sync.dma_start(wt[:],
                                                      wo_d[ts(dc, P), ts(mo, P)])
                                    for qb in range(n_qb):
                                        nc.tensor.matmul(
                                            pss[qb][:], wt[:], CT[:, dc, ts(qb, QB)],
                                            start=(dc == 0), stop=(dc == n_dc - 1))
                                for qb in range(n_qb):
                                    at = p_wos.tile([P, QB], F32, tag="attn_t")
                                    nc.scalar.activation(at[:], pss[qb][:],
                                                         AF.Identity,
                                                         bias=bo_t[:, mo:mo + 1])
                                    nc.vector.tensor_add(XQ[:, mo, ts(qb, QB)],
                                                         XQ[:, mo, ts(qb, QB)],
                                                         at[:])

                if dbg:
                    for dc in range(n_dc):
                        nc.sync.dma_start(dbg_d["dh2"][ts(dc, P), :], XQ[:, dc, :])
                # ---------- Phase 3: LN2 + MLP ----------
                with tc.tile_pool(name="p_mlp", bufs=1) as p_mlp, \
                     tc.tile_pool(name="p_w1", bufs=3) as p_w1, \
                     tc.tile_pool(name="p_w2", bufs=2) as p_w2, \
                     tc.tile_pool(name="p_tmp2", bufs=2) as p_tmp2, \
                     tc.tile_pool(name="p_st2", bufs=1) as p_st2, \
                     tc.tile_pool(name="p_out", bufs=3) as p_out, \
                     tc.tile_pool(name="ps_st2", bufs=2, space="PSUM") as ps_st2, \
                     tc.tile_pool(name="ps_f", bufs=4, space="PSUM") as ps_f:

                    XN2 = p_mlp.tile([P, n_dc, Q], F32R)
                    _layernorm(nc, ones_h, eps_t, p_mlp, p_tmp2, p_st2, ps_st2,
                               lambda dc: XQ[:, dc, :], n_dc, Q, QB,
                               g2_t, be2_t, lambda dc: XN2[:, dc, :])

                    for qb in range(n_qb):
                        qsl = ts(qb, QB)
                        Y1 = p_mlp.tile([P, n_mo, QB], F32R, tag="y1")
                        for mo in range(n_mo):
                            wt = p_w1.tile([P, n_dc, P], F32R, tag="w1")
                            nc.sync.dma_start(
                                wt[:],
                                w1_d[:, ts(mo, P)].rearrange("(c p) m -> p c m", p=P))
                            ps = ps_f.tile([P, QB], F32, tag="ps_f")
                            for dc in range(n_dc):
                                nc.tensor.matmul(ps[:], wt[:, dc, :],
                                                 XN2[:, dc, qsl],
                                                 start=(dc == 0),
                                                 stop=(dc == n_dc - 1))
                            nc.scalar.activation(Y1[:, mo, :], ps[:], AF.Gelu,
                                                 bias=b1_t[:, mo:mo + 1])
                        n_mh = max(1, n_mo // 2)
                        for mo2 in range(n_dc):
                            ps = ps_f.tile([P, QB], F32, tag="ps_f")
                            for half in range(n_mo // n_mh):
                                wt = p_w2.tile([P, n_mh, P], F32R, tag="w2")
                                nc.sync.dma_start(
                                    wt[:],
                                    w2_d[ts(half, n_mh * P), ts(mo2, P)]
                                    .rearrange("(c p) m -> p c m", p=P))
                                for kk in range(n_mh):
                                    kc = half * n_mh + kk
                                    nc.tensor.matmul(ps[:], wt[:, kk, :], Y1[:, kc, :],
                                                     start=(kc == 0),
                                                     stop=(kc == n_mo - 1))
                            ot = p_out.tile([P, QB], F32, tag="out")
                            nc.vector.tensor_add(ot[:], ps[:], XQ[:, mo2, qsl])
                            nc.vector.tensor_scalar_add(ot[:], ot[:],
                                                        b2_t[:, mo2:mo2 + 1])
                            nc.sync.dma_start(yT_d[ts(mo2, P), qsl], ot[:])
    nc.compile()
    return nc


_NC_CACHE = {}


def _get_nc(T, Q, Dm, Hh, Mlp, n_cores):
    key = (T, Q, Dm, Hh, Mlp, n_cores)
    if key not in _NC_CACHE:
        _NC_CACHE[key] = build_bass(T, Q, Dm, Hh, Mlp, n_cores)
    return _NC_CACHE[key]


def make_in_maps(inputs, n_cores):
    """Per-core input dicts for the (batch x seq-half) sharding."""
    x = np.asarray(inputs["x"], np.float32)
    Bq, Sq, Dq = x.shape
    Qtok = Sq * Bq // n_cores
    bf = ml_dtypes.bfloat16
    shared = {
        "g1": np.asarray(inputs["ln1_g"], np.float32),
        "be1": np.asarray(inputs["ln1_b"], np.float32),
        "g2": np.asarray(inputs["ln2_g"], np.float32),
        "be2": np.asarray(inputs["ln2_b"], np.float32),
        "wq16": np.asarray(inputs["Wq"], np.float32).astype(bf),
        "wk16": np.asarray(inputs["Wk"], np.float32).astype(bf),
        "wv16": np.asarray(inputs["Wv"], np.float32).astype(bf),
        "wo16": np.asarray(inputs["Wo"], np.float32).astype(bf),
        "w1r": np.asarray(inputs["W1"], np.float32),
        "w2r": np.asarray(inputs["W2"], np.float32),
        "bq": np.asarray(inputs["bq"], np.float32),
        "bk": np.asarray(inputs["bk"], np.float32),
        "bv": np.asarray(inputs["bv"], np.float32),
        "bo": np.asarray(inputs["bo"], np.float32),
        "b1": np.asarray(inputs["b1"], np.float32),
        "b2": np.asarray(inputs["b2"], np.float32),
        "ones16": np.ones((P, 1), bf),
    }
    in_maps = []
    for c in range(n_cores):
        b = c // (n_cores // Bq)
        qoff = (c % (n_cores // Bq)) * Qtok
        m = dict(shared)
        m["xT"] = np.ascontiguousarray(x[b].T)
        m["xqT"] = np.ascontiguousarray(x[b, qoff:qoff + Qtok].T)
        in_maps.append(m)
    return in_maps, Qtok


def kernel(**inputs):
    x = np.asarray(inputs["x"], np.float32)
    Bq, Sq, Dq = x.shape
    in_maps, Qtok = make_in_maps(inputs, N_CORES)
    nc = _get_nc(Sq, Qtok, Dq, H, MLP, N_CORES)
    res = run_bass_kernel_spmd(nc, in_maps, core_ids=list(range(N_CORES)))
    out = np.empty((Bq, Sq, Dq), np.float32)
    per_b = N_CORES // Bq
    for c in range(N_CORES):
        b = c // per_b
        qoff = (c % per_b) * Qtok
        out[b, qoff:qoff + Qtok, :] = res.results[c]["yT"].T
    return out


# revision 24
# speedup vs baseline: 1.0078x; 1.0078x over previous
"""Trainium2 Bass kernel for a dense transformer block (LN1 -> MHA -> LN2 -> MLP).

Sharding: 8 cores = (batch b in 0..3) x (sequence half in 0..1). Each core
computes the block output for its 1024 query tokens; K/V are computed for the
batch's full 2048 tokens on each core (replicated within the pair), so there
is zero cross-core communication.

Layout: on-chip activations are transposed ([feature, token]) so matmul
chains compose without transposes; the host transposes x per core and
transposes the per-core outputs back.

Dtypes: attention path bf16 (Q/K/V/probs), residuals fp32, MLP float32r
(full PE speed at N>=256, ~1e-4 matmul accuracy), LN stats via bf16 PE
ones-matmuls (rounding noise averages out across 1024 terms).
"""

import sys

if '/opt/trn_rl_repo' not in sys.path:
    sys.path.insert(0, '/opt/trn_rl_repo')

import numpy as np
import ml_dtypes

import concourse.tile as tile
import concourse.mybir as mybir
from concourse import bacc
from concourse.bass import ts
from concourse.bass_utils import run_bass_kernel_spmd

P = 128
F32 = mybir.dt.float32
F32R = mybir.dt.float32r
BF16 = mybir.dt.bfloat16
AF = mybir.ActivationFunctionType
EPS = 1e-6

B, S, D, H, MLP = 4, 2048, 1024, 16, 4096
N_CORES = 8


def _layernorm(nc, ones_h, eps_t, p_bc, p_tmp, p_st, ps_st, src_fn, n_dc, Tn, TBn,
               g_t, b_t, out_fn, dram_src=None):
    """LayerNorm along the feature (partition-chunk) direction.

    src_fn(dc) -> [P, Tn] fp32 AP of a resident tile, or None with dram_src
    set to a [Dm, Tn] fp32 dram AP to stream chunks (two passes over dram).
    out_fn(dc) -> [P, Tn] dest AP (any dtype).
    Feature sums via PE ones-matmuls on bf16 casts.
    """
    n_tb = Tn // TBn
    inv_d = 1.0 / (n_dc * P)
    if dram_src is None:
        mean_bc_full = p_bc.tile([P, Tn], F32, tag="ln_meanbc")
        rstd_bc_full = p_bc.tile([P, Tn], F32, tag="ln_rstdbc")
    for tb in range(n_tb):
        sl = ts(tb, TBn)
        ps_m = ps_st.tile([1, TBn], F32, tag="ps_stat")
        ps_s = ps_st.tile([1, TBn], F32, tag="ps_stat")
        for dc in range(n_dc):
            st, sp = (dc == 0), (dc == n_dc - 1)
            if dram_src is not None:
                xc = p_tmp.tile([P, TBn], F32, tag="ln_xc")
                nc.sync.dma_start(xc[:], dram_src[ts(dc, P), sl])
                src_sl = xc[:]
            else:
                src_sl = src_fn(dc)[:, sl]
            xb = p_tmp.tile([P, TBn], BF16, tag="ln_xb")
            nc.vector.tensor_copy(xb[:], src_sl)
            nc.tensor.matmul(ps_m[:], ones_h[:], xb[:], start=st, stop=sp)
            xsq = p_tmp.tile([P, TBn], BF16, tag="ln_xsq")
            nc.scalar.activation(xsq[:], src_sl, AF.Square)
            nc.tensor.matmul(ps_s[:], ones_h[:], xsq[:], start=st, stop=sp)
        mean = p_st.tile([1, TBn], F32)
        nc.vector.tensor_scalar_mul(mean[:], ps_m[:], inv_d)
        ex2 = p_st.tile([1, TBn], F32)
        nc.vector.tensor_scalar_mul(ex2[:], ps_s[:], inv_d)
        var = p_st.tile([1, TBn], F32)
        nc.vector.tensor_mul(var[:], mean[:], mean[:])
        nc.vector.tensor_sub(var[:], ex2[:], var[:])
        std = p_st.tile([1, TBn], F32)
        nc.scalar.activation(std[:], var[:], AF.Sqrt, bias=eps_t[:, :])
        rstd = p_st.tile([1, TBn], F32)
        nc.vector.reciprocal(rstd[:], std[:])
        if dram_src is None:
            nc.gpsimd.partition_broadcast(mean_bc_full[:, sl], mean[:])
            nc.gpsimd.partition_broadcast(rstd_bc_full[:, sl], rstd[:])
        else:
            # chunked apply: broadcast per token-block, re-stream source
            mean_bc = p_tmp.tile([P, TBn], F32, tag="ln_meanbc_c")
            rstd_bc = p_tmp.tile([P, TBn], F32, tag="ln_rstdbc_c")
            nc.gpsimd.partition_broadcast(mean_bc[:], mean[:])
            nc.gpsimd.partition_broadcast(rstd_bc[:], rstd[:])
            for dc in range(n_dc):
                t0 = p_tmp.tile([P, TBn], F32, tag="ln_xa")
                nc.sync.dma_start(t0[:], dram_src[ts(dc, P), sl])
                nc.vector.tensor_sub(t0[:], t0[:], mean_bc[:])
                nc.vector.tensor_mul(t0[:], t0[:], rstd_bc[:])
                nc.scalar.activation(out_fn(dc)[:, sl], t0[:], AF.Identity,
                                     bias=b_t[:, dc:dc + 1],
                                     scale=g_t[:, dc:dc + 1])
    if dram_src is None:
        for dc in range(n_dc):
            t0 = p_tmp.tile([P, Tn], F32, tag="ln_t0")
            nc.vector.tensor_sub(t0[:], src_fn(dc), mean_bc_full[:])
            nc.vector.tensor_mul(t0[:], t0[:], rstd_bc_full[:])
            nc.scalar.activation(out_fn(dc), t0[:], AF.Identity,
                                 bias=b_t[:, dc:dc + 1], scale=g_t[:, dc:dc + 1])


def build_bass(T, Q, Dm, Hh, Mlp, n_cores, dbg=False):
    dh = Dm // Hh
    assert dh == 64, "head packing assumes DH=64"
    n_dc = Dm // P
    n_tk = T // P
    TB = min(512, T)
    n_tb = T // TB
    QB = min(512, Q)
    n_qb = Q // QB
    QQ = min(512, Q)
    n_qq = Q // QQ
    n_mo = Mlp // P
    n_hp = Hh // 2

    nc = bacc.Bacc("TRN2", target_bir_lowering=False, debug=False,
                   enable_asserts=False, num_devices=n_cores)

    def din(name, shape, dt):
        return nc.dram_tensor(name, shape, dt, kind="ExternalInput").ap()

    xT_d = din("xT", (Dm, T), F32)
    xqT_d = din("xqT", (Dm, Q), F32)
    g1_d, be1_d = din("g1", (Dm,), F32), din("be1", (Dm,), F32)
    g2_d, be2_d = din("g2", (Dm,), F32), din("be2", (Dm,), F32)
    wq_d, wk_d = din("wq16", (Dm, Dm), BF16), din("wk16", (Dm, Dm), BF16)
    wv_d, wo_d = din("wv16", (Dm, Dm), BF16), din("wo16", (Dm, Dm), BF16)
    w1_d = din("w1r", (Dm, Mlp), F32R)
    w2_d = din("w2r", (Mlp, Dm), F32R)
    bq_d, bk_d = din("bq", (Dm,), F32), din("bk", (Dm,), F32)
    bv_d, bo_d = din("bv", (Dm,), F32), din("bo", (Dm,), F32)
    b1_d, b2_d = din("b1", (Mlp,), F32), din("b2", (Dm,), F32)
    ones_d = din("ones16", (P, 1), BF16)
    yT_d = nc.dram_tensor("yT", (Dm, Q), F32, kind="ExternalOutput").ap()
    dbg_d = {}
    if dbg:
        for nm, shape, dt in [("dXN", (Dm, T), BF16), ("dXNQ", (Dm, Q), BF16),
                              ("dKT", (Dm, T), BF16), ("dQT", (Dm, Q), BF16),
                              ("dVT", (T, Dm), BF16), ("dCT", (Dm, Q), BF16),
                              ("dh2", (Dm, Q), F32), ("drbc", (P, Q), F32),
                              ("dexp", (T, Q), BF16)]:
            dbg_d[nm] = nc.dram_tensor(nm, shape, dt, kind="ExternalOutput").ap()

    with tile.TileContext(nc) as tc:
        with tc.tile_pool(name="const", bufs=1) as constp:
            ones_h = constp.tile([P, 1], BF16)
            nc.sync.dma_start(ones_h[:], ones_d[:, :])
            eps_t = constp.tile([1, 1], F32)
            nc.vector.memset(eps_t[:], EPS)
            ones_f = constp.tile([P, P], BF16)
            nc.vector.memset(ones_f[:], 1.0)

            def vec_tile(src, n, nm):
                t = constp.tile([P, n], F32, tag=nm, name=nm)
                nc.sync.dma_start(t[:], src.rearrange("(c p) -> p c", p=P))
                return t

            g1_t, be1_t = vec_tile(g1_d, n_dc, "g1"), vec_tile(be1_d, n_dc, "be1")
            g2_t, be2_t = vec_tile(g2_d, n_dc, "g2"), vec_tile(be2_d, n_dc, "be2")
            bq_t, bk_t = vec_tile(bq_d, n_dc, "bq"), vec_tile(bk_d, n_dc, "bk")
            bo_t, b2_t = vec_tile(bo_d, n_dc, "bo"), vec_tile(b2_d, n_dc, "b2")
            b1_t = vec_tile(b1_d, n_mo, "b1")
            # bv broadcast along free dim (V is [token, d_out])
            bv_row = constp.tile([1, Dm], F32)
            nc.sync.dma_start(bv_row[:, :], bv_d[None, :])
            bv_bc = constp.tile([P, Dm], F32)
            nc.gpsimd.partition_broadcast(bv_bc[:], bv_row[:])

            with tc.tile_pool(name="p_h2", bufs=1) as p_h2:
                XQ = p_h2.tile([P, n_dc, Q], F32)  # x_q, becomes h2

                with tc.tile_pool(name="p_kv", bufs=1) as p_kv:
                    KT = p_kv.tile([P, n_dc, T], BF16)
                    VT = p_kv.tile([P, n_tk, Dm], BF16)
                    QT = p_kv.tile([P, n_dc, Q], BF16)

                    # ---------- Phase 1: LN1 + QKV ----------
                    with tc.tile_pool(name="p_act", bufs=1) as p_act, \
                         tc.tile_pool(name="p_str", bufs=4) as p_str, \
                         tc.tile_pool(name="p_tmp", bufs=2) as p_tmp, \
                         tc.tile_pool(name="p_st", bufs=1) as p_st, \
                         tc.tile_pool(name="ps_st", bufs=2, space="PSUM") as ps_st, \
                         tc.tile_pool(name="ps_mm", bufs=6, space="PSUM") as ps_mm:

                        XN = p_act.tile([P, n_dc, T], BF16)
                        _layernorm(nc, ones_h, eps_t, p_act, p_tmp, p_st, ps_st,
                                   None, n_dc, T, TB,
                                   g1_t, be1_t, lambda dc: XN[:, dc, :],
                                   dram_src=xT_d)
                        XNQ = p_act.tile([P, n_dc, Q], BF16)
                        _layernorm(nc, ones_h, eps_t, p_act, p_tmp, p_st, ps_st,
                                   None, n_dc, Q, QB,
                                   g1_t, be1_t, lambda dc: XNQ[:, dc, :],
                                   dram_src=xqT_d)

                        if dbg:
                            for dc in range(n_dc):
                                nc.sync.dma_start(dbg_d["dXN"][ts(dc, P), :], XN[:, dc, :])
                                nc.sync.dma_start(dbg_d["dXNQ"][ts(dc, P), :], XNQ[:, dc, :])
                        # K^T: lhsT = Wk chunk, rhs = XN
                        for mo in range(n_dc):
                            pss = [ps_mm.tile([P, TB], F32, tag="ps_mm", name="ps_mm")
                                   for _ in range(n_tb)]
                            for dc in range(n_dc):
                                wt = p_str.tile([P, P], BF16, tag="wkq")
                                nc.sync.dma_start(wt[:], wk_d[ts(dc, P), ts(mo, P)])
                                for tb in range(n_tb):
                                    nc.tensor.matmul(
                                        pss[tb][:], wt[:], XN[:, dc, ts(tb, TB)],
                                        start=(dc == 0), stop=(dc == n_dc - 1))
                            for tb in range(n_tb):
                                nc.scalar.activation(KT[:, mo, ts(tb, TB)],
                                                     pss[tb][:], AF.Identity,
                                                     bias=bk_t[:, mo:mo + 1])
                        # Q^T from XNQ
                        for mo in range(n_dc):
                            pss = [ps_mm.tile([P, QB], F32, tag="ps_mm", name="ps_mm")
                                   for _ in range(n_qb)]
                            for dc in range(n_dc):
                                wt = p_str.tile([P, P], BF16, tag="wkq")
                                nc.sync.dma_start(wt[:], wq_d[ts(dc, P), ts(mo, P)])
                                for qb in range(n_qb):
                                    nc.tensor.matmul(
                                        pss[qb][:], wt[:], XNQ[:, dc, ts(qb, QB)],
                                        start=(dc == 0), stop=(dc == n_dc - 1))
                            for qb in range(n_qb):
                                nc.scalar.activation(QT[:, mo, ts(qb, QB)],
                                                     pss[qb][:], AF.Identity,
                                                     bias=bq_t[:, mo:mo + 1])
                        # V: lhsT = XN chunk (tokens as M), rhs = Wv streamed
                        # per token-group (re-read n_tk/TG times)
                        NO = min(TB, Dm)
                        n_no = Dm // NO
                        TG = 4
                        for tg in range(0, n_tk, TG):
                            tos = range(tg, min(tg + TG, n_tk))
                            for no in range(n_no):
                                pss = [ps_mm.tile([P, NO], F32, tag="ps_mm",
                                                  name="ps_mm") for _ in tos]
                                for dc in range(n_dc):
                                    wvt = p_str.tile([P, NO], BF16, tag="wv")
                                    nc.sync.dma_start(
                                        wvt[:], wv_d[ts(dc, P), ts(no, NO)])
                                    for i, to in enumerate(tos):
                                        nc.tensor.matmul(
                                            pss[i][:], XN[:, dc, ts(to, P)],
                                            wvt[:],
                                            start=(dc == 0), stop=(dc == n_dc - 1))
                                for i, to in enumerate(tos):
                                    nc.vector.tensor_add(VT[:, to, ts(no, NO)],
                                                         pss[i][:],
                                                         bv_bc[:, ts(no, NO)])

                    if dbg:
                        for dc in range(n_dc):
                            nc.sync.dma_start(dbg_d["dKT"][ts(dc, P), :], KT[:, dc, :])
                            nc.sync.dma_start(dbg_d["dQT"][ts(dc, P), :], QT[:, dc, :])
                        for to in range(n_tk):
                            nc.sync.dma_start(dbg_d["dVT"][ts(to, P), :], VT[:, to, :])
                    # ---------- Phase 2: attention ----------
                    with tc.tile_pool(name="p_attn", bufs=1) as p_attn:
                        CT = p_attn.tile([P, n_dc, Q], BF16)
                        for dc in range(n_dc):
                            nc.sync.dma_start(XQ[:, dc, :], xqT_d[ts(dc, P), :])
                        with tc.tile_pool(name="p_exp", bufs=3) as p_exp, \
                             tc.tile_pool(name="p_rb", bufs=3) as p_rb, \
                             tc.tile_pool(name="ps_sc", bufs=4, space="PSUM") as ps_sc, \
                             tc.tile_pool(name="ps_ctx", bufs=2, space="PSUM") as ps_ctx, \
                             tc.tile_pool(name="ps_dn", bufs=2, space="PSUM") as ps_dn, \
                             tc.tile_pool(name="ps_rb", bufs=2, space="PSUM") as ps_rb:
                            for qq in range(n_qq):
                                qsl = ts(qq, QQ)
                                for hp in range(n_hp):
                                    exps = [p_exp.tile([P, n_tk, QQ], BF16,
                                                       tag="expT", name="expT")
                                            for _ in range(2)]
                                    # interleave the two heads' score matmuls:
                                    # they hit different PE row-strips (0/64)
                                    # and run concurrently in the array
                                    for kc in range(n_tk):
                                        for hi in range(2):
                                            r0 = hi * 64
                                            ps_s = ps_sc.tile([P, QQ], F32,
                                                              tag="ps_s")
                                            nc.tensor.matmul(
                                                ps_s[:],
                                                KT[r0:r0 + 64, hp, ts(kc, P)],
                                                QT[r0:r0 + 64, hp, qsl],
                                                start=True, stop=True)
                                            nc.scalar.activation(
                                                exps[hi][:, kc, :], ps_s[:],
                                                AF.Exp, scale=0.125)
                                    rbcs = []
                                    dns = [ps_dn.tile([P, QQ], F32, tag="ps_d",
                                                      name="ps_d")
                                           for _ in range(2)]
                                    for kc in range(n_tk):
                                        for hi in range(2):
                                            nc.tensor.matmul(
                                                dns[hi][:], ones_f[:],
                                                exps[hi][:, kc, :],
                                                start=(kc == 0),
                                                stop=(kc == n_tk - 1))
                                    for hi in range(2):
                                        rbc_h = p_rb.tile([P, QQ], F32, tag="rbc",
                                                          name="rbc")
                                        nc.vector.reciprocal(rbc_h[:], dns[hi][:])
                                        rbcs.append(rbc_h)
                                    if dbg and hp == 0:
                                        nc.sync.dma_start(dbg_d["drbc"][0:64, qsl], rbcs[0][0:64, :])
                                        nc.sync.dma_start(dbg_d["drbc"][64:128, qsl], rbcs[1][64:128, :])
                                        for kc in range(n_tk):
                                            nc.sync.dma_start(
                                                dbg_d["dexp"][ts(kc, P), qsl],
                                                exps[0][:, kc, :])
                                    # interleaved ctx matmuls hit different PE
                                    # col-strips (0/64) -> concurrent
                                    ps_c = ps_ctx.tile([P, QQ], F32, tag="ps_c")
                                    for kc in range(n_tk):
                                        for hi in range(2):
                                            h = 2 * hp + hi
                                            nc.tensor.matmul(
                                                ps_c[hi * 64:hi * 64 + 64, :],
                                                VT[:, kc, ts(h, 64)],
                                                exps[hi][:, kc, :],
                                                start=(kc == 0),
                                                stop=(kc == n_tk - 1),
                                                tile_position=(0, hi * 64))
                                    for hi in range(2):
                                        r0 = hi * 64
                                        nc.vector.tensor_mul(
                                            CT[r0:r0 + 64, hp, qsl],
                                            ps_c[r0:r0 + 64, :],
                                            rbcs[hi][r0:r0 + 64, :])

                        if dbg:
                            for dc in range(n_dc):
                                nc.sync.dma_start(dbg_d["dCT"][ts(dc, P), :], CT[:, dc, :])
                        # Wo + bias + residual into XQ
                        with tc.tile_pool(name="p_wos", bufs=4) as p_wos, \
                             tc.tile_pool(name="ps_wo", bufs=4, space="PSUM") as ps_wo:
                            for mo in range(n_dc):
                                pss = [ps_wo.tile([P, QB], F32, tag="ps_wo", name="ps_wo")
                                       for _ in range(n_qb)]
                                for dc in range(n_dc):
                                    wt = p_wos.tile([P, P], BF16, tag="wo")
                                    nc.sync.dma_start(wt[:],
                                                      wo_d[ts(dc, P), ts(mo, P)])
                                    for qb in range(n_qb):
                                        nc.tensor.matmul(
                                            pss[qb][:], wt[:], CT[:, dc, ts(qb, QB)],
                                            start=(dc == 0), stop=(dc == n_dc - 1))
                                for qb in range(n_qb):
                                    at = p_wos.tile([P, QB], F32, tag="attn_t")
                                    nc.scalar.activation(at[:], pss[qb][:],
                                                         AF.Identity,
                                                         bias=bo_t[:, mo:mo + 1])
                                    nc.vector.tensor_add(XQ[:, mo, ts(qb, QB)],
                                                         XQ[:, mo, ts(qb, QB)],
                                                         at[:])

                if dbg:
                    for dc in range(n_dc):
                        nc.sync.dma_start(dbg_d["dh2"][ts(dc, P), :], XQ[:, dc, :])
                # ---------- Phase 3: LN2 + MLP ----------
                with tc.tile_pool(name="p_mlp", bufs=1) as p_mlp, \
                     tc.tile_pool(name="p_w1", bufs=3) as p_w1, \
                     tc.tile_pool(name="p_w2", bufs=2) as p_w2, \
                     tc.tile_pool(name="p_tmp2", bufs=2) as p_tmp2, \
                     tc.tile_pool(name="p_st2", bufs=1) as p_st2, \
                     tc.tile_pool(name="p_out", bufs=3) as p_out, \
                     tc.tile_pool(name="ps_st2", bufs=2, space="PSUM") as ps_st2, \
                     tc.tile_pool(name="ps_f", bufs=4, space="PSUM") as ps_f:

                    XN2 = p_mlp.tile([P, n_dc, Q], F32R)
                    _layernorm(nc, ones_h, eps_t, p_mlp, p_tmp2, p_st2, ps_st2,
                               lambda dc: XQ[:, dc, :], n_dc, Q, QB,
                               g2_t, be2_t, lambda dc: XN2[:, dc, :])

                    for qb in range(n_qb):
                        qsl = ts(qb, QB)
                        Y1 = p_mlp.tile([P, n_mo, QB], F32R, tag="y1")
                        for mo in range(n_mo):
                            wt = p_w1.tile([P, n_dc, P], F32R, tag="w1")
                            nc.sync.dma_start(
                                wt[:],
                                w1_d[:, ts(mo, P)].rearrange("(c p) m -> p c m", p=P))
                            ps = ps_f.tile([P, QB], F32, tag="ps_f")
                            for dc in range(n_dc):
                                nc.tensor.matmul(ps[:], wt[:, dc, :],
                                                 XN2[:, dc, qsl],
                                                 start=(dc == 0),
                                                 stop=(dc == n_dc - 1))
                            nc.scalar.activation(Y1[:, mo, :], ps[:], AF.Gelu,
                                                 bias=b1_t[:, mo:mo + 1])
                        n_mh = max(1, n_mo // 2)
                        for mo2 in range(n_dc):
                            ps = ps_f.tile([P, QB], F32, tag="ps_f")
                            for half in range(n_mo // n_mh):
                                wt = p_w2.tile([P, n_mh, P], F32R, tag="w2")
                                nc.sync.dma_start(
                                    wt[:],
                                    w2_d[ts(half, n_mh * P), ts(mo2, P)]
                                    .rearrange("(c p) m -> p c m", p=P))
                                for kk in range(n_mh):
                                    kc = half * n_mh + kk
                                    nc.tensor.matmul(ps[:], wt[:, kk, :], Y1[:, kc, :],
                                                     start=(kc == 0),
                                                     stop=(kc == n_mo - 1))
                            ot = p_out.tile([P, QB], F32, tag="out")
                            nc.vector.tensor_add(ot[:], ps[:], XQ[:, mo2, qsl])
                            nc.vector.tensor_scalar_add(ot[:], ot[:],
                                                        b2_t[:, mo2:mo2 + 1])
                            nc.sync.dma_start(yT_d[ts(mo2, P), qsl], ot[:])
    nc.compile()
    return nc


_NC_CACHE = {}


def _get_nc(T, Q, Dm, Hh, Mlp, n_cores):
    key = (T, Q, Dm, Hh, Mlp, n_cores)
    if key not in _NC_CACHE:
        _NC_CACHE[key] = build_bass(T, Q, Dm, Hh, Mlp, n_cores)
    return _NC_CACHE[key]


def make_in_maps(inputs, n_cores):
    """Per-core input dicts for the (batch x seq-half) sharding."""
    x = np.asarray(inputs["x"], np.float32)
    Bq, Sq, Dq = x.shape
    Qtok = Sq * Bq // n_cores
    bf = ml_dtypes.bfloat16
    shared = {
        "g1": np.asarray(inputs["ln1_g"], np.float32),
        "be1": np.asarray(inputs["ln1_b"], np.float32),
        "g2": np.asarray(inputs["ln2_g"], np.float32),
        "be2": np.asarray(inputs["ln2_b"], np.float32),
        "wq16": np.asarray(inputs["Wq"], np.float32).astype(bf),
        "wk16": np.asarray(inputs["Wk"], np.float32).astype(bf),
        "wv16": np.asarray(inputs["Wv"], np.float32).astype(bf),
        "wo16": np.asarray(inputs["Wo"], np.float32).astype(bf),
        "w1r": np.asarray(inputs["W1"], np.float32),
        "w2r": np.asarray(inputs["W2"], np.float32),
        "bq": np.asarray(inputs["bq"], np.float32),
        "bk": np.asarray(inputs["bk"], np.float32),
        "bv": np.asarray(inputs["bv"], np.float32),
        "bo": np.asarray(inputs["bo"], np.float32),
        "b1": np.asarray(inputs["b1"], np.float32),
        "b2": np.asarray(inputs["b2"], np.float32),
        "ones16": np.ones((P, 1), bf),
    }
    in_maps = []
    for c in range(n_cores):
        b = c // (n_cores // Bq)
        qoff = (c % (n_cores // Bq)) * Qtok
        m = dict(shared)
        m["xT"] = np.ascontiguousarray(x[b].T)
        m["xqT"] = np.ascontiguousarray(x[b, qoff:qoff + Qtok].T)
        in_maps.append(m)
    return in_maps, Qtok


def kernel(**inputs):
    x = np.asarray(inputs["x"], np.float32)
    Bq, Sq, Dq = x.shape
    in_maps, Qtok = make_in_maps(inputs, N_CORES)
    nc = _get_nc(Sq, Qtok, Dq, H, MLP, N_CORES)
    res = run_bass_kernel_spmd(nc, in_maps, core_ids=list(range(N_CORES)))
    out = np.empty((Bq, Sq, Dq), np.float32)
    per_b = N_CORES // Bq
    for c in range(N_CORES):
        b = c // per_b
        qoff = (c % per_b) * Qtok
        out[b, qoff:qoff + Qtok, :] = res.results[c]["yT"].T
    return out


# revision 26
# speedup vs baseline: 1.0394x; 1.0313x over previous
"""Trainium2 Bass kernel for a dense transformer block (LN1 -> MHA -> LN2 -> MLP).

Sharding: 8 cores = (batch b in 0..3) x (sequence half in 0..1). Each core
computes the block output for its 1024 query tokens; K/V are computed for the
batch's full 2048 tokens on each core (replicated within the pair), so there
is zero cross-core communication.

Layout: on-chip activations are transposed ([feature, token]) so matmul
chains compose without transposes; the host transposes x per core and
transposes the per-core outputs back.

Dtypes: attention path bf16 (Q/K/V/probs), residuals fp32, MLP float32r
(full PE speed at N>=256, ~1e-4 matmul accuracy), LN stats via bf16 PE
ones-matmuls (rounding noise averages out across 1024 terms).
"""

import sys

if '/opt/trn_rl_repo' not in sys.path:
    sys.path.insert(0, '/opt/trn_rl_repo')

import numpy as np
import ml_dtypes

import concourse.tile as tile
import concourse.mybir as mybir
from concourse import bacc
from concourse.bass import ts
from concourse.bass_utils import run_bass_kernel_spmd

P = 128
F32 = mybir.dt.float32
F32R = mybir.dt.float32r
BF16 = mybir.dt.bfloat16
AF = mybir.ActivationFunctionType
EPS = 1e-6

B, S, D, H, MLP = 4, 2048, 1024, 16, 4096
N_CORES = 8


def _layernorm(nc, ones_h, eps_t, p_bc, p_tmp, p_st, ps_st, src_fn, n_dc, Tn, TBn,
               g_t, b_t, out_fn, dram_src=None):
    """LayerNorm along the feature (partition-chunk) direction.

    src_fn(dc) -> [P, Tn] fp32 AP of a resident tile, or None with dram_src
    set to a [Dm, Tn] fp32 dram AP to stream chunks (two passes over dram).
    out_fn(dc) -> [P, Tn] dest AP (any dtype).
    Feature sums via PE ones-matmuls on bf16 casts.
    """
    n_tb = Tn // TBn
    inv_d = 1.0 / (n_dc * P)
    if dram_src is None:
        mean_bc_full = p_bc.tile([P, Tn], F32, tag="ln_meanbc")
        rstd_bc_full = p_bc.tile([P, Tn], F32, tag="ln_rstdbc")
    for tb in range(n_tb):
        sl = ts(tb, TBn)
        ps_m = ps_st.tile([1, TBn], F32, tag="ps_stat")
        ps_s = ps_st.tile([1, TBn], F32, tag="ps_stat")
        for dc in range(n_dc):
            st, sp = (dc == 0), (dc == n_dc - 1)
            if dram_src is not None:
                xc = p_tmp.tile([P, TBn], F32, tag="ln_xc")
                nc.sync.dma_start(xc[:], dram_src[ts(dc, P), sl])
                src_sl = xc[:]
            else:
                src_sl = src_fn(dc)[:, sl]
            xb = p_tmp.tile([P, TBn], BF16, tag="ln_xb")
            nc.vector.tensor_copy(xb[:], src_sl)
            nc.tensor.matmul(ps_m[:], ones_h[:], xb[:], start=st, stop=sp)
            xsq = p_tmp.tile([P, TBn], BF16, tag="ln_xsq")
            nc.scalar.activation(xsq[:], src_sl, AF.Square)
            nc.tensor.matmul(ps_s[:], ones_h[:], xsq[:], start=st, stop=sp)
        mean = p_st.tile([1, TBn], F32)
        nc.vector.tensor_scalar_mul(mean[:], ps_m[:], inv_d)
        ex2 = p_st.tile([1, TBn], F32)
        nc.vector.tensor_scalar_mul(ex2[:], ps_s[:], inv_d)
        var = p_st.tile([1, TBn], F32)
        nc.vector.tensor_mul(var[:], mean[:], mean[:])
        nc.vector.tensor_sub(var[:], ex2[:], var[:])
        std = p_st.tile([1, TBn], F32)
        nc.scalar.activation(std[:], var[:], AF.Sqrt, bias=eps_t[:, :])
        rstd = p_st.tile([1, TBn], F32)
        nc.vector.reciprocal(rstd[:], std[:])
        if dram_src is None:
            nc.gpsimd.partition_broadcast(mean_bc_full[:, sl], mean[:])
            nc.gpsimd.partition_broadcast(rstd_bc_full[:, sl], rstd[:])
        else:
            # chunked apply: broadcast per token-block, re-stream source
            mean_bc = p_tmp.tile([P, TBn], F32, tag="ln_meanbc_c")
            rstd_bc = p_tmp.tile([P, TBn], F32, tag="ln_rstdbc_c")
            nc.gpsimd.partition_broadcast(mean_bc[:], mean[:])
            nc.gpsimd.partition_broadcast(rstd_bc[:], rstd[:])
            for dc in range(n_dc):
                t0 = p_tmp.tile([P, TBn], F32, tag="ln_xa")
                nc.sync.dma_start(t0[:], dram_src[ts(dc, P), sl])
                nc.vector.tensor_sub(t0[:], t0[:], mean_bc[:])
                nc.vector.tensor_mul(t0[:], t0[:], rstd_bc[:])
                nc.scalar.activation(out_fn(dc)[:, sl], t0[:], AF.Identity,
                                     bias=b_t[:, dc:dc + 1],
                                     scale=g_t[:, dc:dc + 1])
    if dram_src is None:
        for dc in range(n_dc):
            t0 = p_tmp.tile([P, Tn], F32, tag="ln_t0")
            nc.vector.tensor_sub(t0[:], src_fn(dc), mean_bc_full[:])
            nc.vector.tensor_mul(t0[:], t0[:], rstd_bc_full[:])
            nc.scalar.activation(out_fn(dc), t0[:], AF.Identity,
                                 bias=b_t[:, dc:dc + 1], scale=g_t[:, dc:dc + 1])


def build_bass(T, Q, Dm, Hh, Mlp, n_cores, dbg=False):
    dh = Dm // Hh
    assert dh == 64, "head packing assumes DH=64"
    n_dc = Dm // P
    n_tk = T // P
    TB = min(512, T)
    n_tb = T // TB
    QB = min(512, Q)
    n_qb = Q // QB
    QQ = min(512, Q)
    n_qq = Q // QQ
    n_mo = Mlp // P
    n_hp = Hh // 2

    nc = bacc.Bacc("TRN2", target_bir_lowering=False, debug=False,
                   enable_asserts=False, num_devices=n_cores)

    def din(name, shape, dt):
        return nc.dram_tensor(name, shape, dt, kind="ExternalInput").ap()

    xT_d = din("xT", (Dm, T), F32)
    xqT_d = din("xqT", (Dm, Q), F32)
    g1_d, be1_d = din("g1", (Dm,), F32), din("be1", (Dm,), F32)
    g2_d, be2_d = din("g2", (Dm,), F32), din("be2", (Dm,), F32)
    wq_d, wk_d = din("wq16", (Dm, Dm), BF16), din("wk16", (Dm, Dm), BF16)
    wv_d, wo_d = din("wv16", (Dm, Dm), BF16), din("wo16", (Dm, Dm), BF16)
    w1_d = din("w1r", (Dm, Mlp), F32R)
    w2_d = din("w2r16", (Mlp, Dm), BF16)
    bq_d, bk_d = din("bq", (Dm,), F32), din("bk", (Dm,), F32)
    bv_d, bo_d = din("bv", (Dm,), F32), din("bo", (Dm,), F32)
    b1_d, b2_d = din("b1", (Mlp,), F32), din("b2", (Dm,), F32)
    ones_d = din("ones16", (P, 1), BF16)
    yT_d = nc.dram_tensor("yT", (Dm, Q), F32, kind="ExternalOutput").ap()
    dbg_d = {}
    if dbg:
        for nm, shape, dt in [("dXN", (Dm, T), BF16), ("dXNQ", (Dm, Q), BF16),
                              ("dKT", (Dm, T), BF16), ("dQT", (Dm, Q), BF16),
                              ("dVT", (T, Dm), BF16), ("dCT", (Dm, Q), BF16),
                              ("dh2", (Dm, Q), F32), ("drbc", (P, Q), F32),
                              ("dexp", (T, Q), BF16)]:
            dbg_d[nm] = nc.dram_tensor(nm, shape, dt, kind="ExternalOutput").ap()

    with tile.TileContext(nc) as tc:
        with tc.tile_pool(name="const", bufs=1) as constp:
            ones_h = constp.tile([P, 1], BF16)
            nc.sync.dma_start(ones_h[:], ones_d[:, :])
            eps_t = constp.tile([1, 1], F32)
            nc.vector.memset(eps_t[:], EPS)
            ones_f = constp.tile([P, P], BF16)
            nc.vector.memset(ones_f[:], 1.0)

            def vec_tile(src, n, nm):
                t = constp.tile([P, n], F32, tag=nm, name=nm)
                nc.sync.dma_start(t[:], src.rearrange("(c p) -> p c", p=P))
                return t

            g1_t, be1_t = vec_tile(g1_d, n_dc, "g1"), vec_tile(be1_d, n_dc, "be1")
            g2_t, be2_t = vec_tile(g2_d, n_dc, "g2"), vec_tile(be2_d, n_dc, "be2")
            bq_t, bk_t = vec_tile(bq_d, n_dc, "bq"), vec_tile(bk_d, n_dc, "bk")
            bo_t, b2_t = vec_tile(bo_d, n_dc, "bo"), vec_tile(b2_d, n_dc, "b2")
            b1_t = vec_tile(b1_d, n_mo, "b1")
            # bv broadcast along free dim (V is [token, d_out])
            bv_row = constp.tile([1, Dm], F32)
            nc.sync.dma_start(bv_row[:, :], bv_d[None, :])
            bv_bc = constp.tile([P, Dm], F32)
            nc.gpsimd.partition_broadcast(bv_bc[:], bv_row[:])

            with tc.tile_pool(name="p_h2", bufs=1) as p_h2:
                XQ = p_h2.tile([P, n_dc, Q], F32)  # x_q, becomes h2

                with tc.tile_pool(name="p_kv", bufs=1) as p_kv:
                    KT = p_kv.tile([P, n_dc, T], BF16)
                    VT = p_kv.tile([P, n_tk, Dm], BF16)
                    QT = p_kv.tile([P, n_dc, Q], BF16)

                    # ---------- Phase 1: LN1 + QKV ----------
                    with tc.tile_pool(name="p_act", bufs=1) as p_act, \
                         tc.tile_pool(name="p_str", bufs=4) as p_str, \
                         tc.tile_pool(name="p_tmp", bufs=2) as p_tmp, \
                         tc.tile_pool(name="p_st", bufs=1) as p_st, \
                         tc.tile_pool(name="ps_st", bufs=2, space="PSUM") as ps_st, \
                         tc.tile_pool(name="ps_mm", bufs=6, space="PSUM") as ps_mm:

                        XN = p_act.tile([P, n_dc, T], BF16)
                        _layernorm(nc, ones_h, eps_t, p_act, p_tmp, p_st, ps_st,
                                   None, n_dc, T, TB,
                                   g1_t, be1_t, lambda dc: XN[:, dc, :],
                                   dram_src=xT_d)
                        XNQ = p_act.tile([P, n_dc, Q], BF16)
                        _layernorm(nc, ones_h, eps_t, p_act, p_tmp, p_st, ps_st,
                                   None, n_dc, Q, QB,
                                   g1_t, be1_t, lambda dc: XNQ[:, dc, :],
                                   dram_src=xqT_d)

                        if dbg:
                            for dc in range(n_dc):
                                nc.sync.dma_start(dbg_d["dXN"][ts(dc, P), :], XN[:, dc, :])
                                nc.sync.dma_start(dbg_d["dXNQ"][ts(dc, P), :], XNQ[:, dc, :])
                        # K^T: lhsT = Wk chunk, rhs = XN
                        for mo in range(n_dc):
                            pss = [ps_mm.tile([P, TB], F32, tag="ps_mm", name="ps_mm")
                                   for _ in range(n_tb)]
                            for dc in range(n_dc):
                                wt = p_str.tile([P, P], BF16, tag="wkq")
                                nc.sync.dma_start(wt[:], wk_d[ts(dc, P), ts(mo, P)])
                                for tb in range(n_tb):
                                    nc.tensor.matmul(
                                        pss[tb][:], wt[:], XN[:, dc, ts(tb, TB)],
                                        start=(dc == 0), stop=(dc == n_dc - 1))
                            for tb in range(n_tb):
                                nc.vector.tensor_scalar_add(KT[:, mo, ts(tb, TB)],
                                                            pss[tb][:],
                                                            bk_t[:, mo:mo + 1])
                        # Q^T from XNQ
                        for mo in range(n_dc):
                            pss = [ps_mm.tile([P, QB], F32, tag="ps_mm", name="ps_mm")
                                   for _ in range(n_qb)]
                            for dc in range(n_dc):
                                wt = p_str.tile([P, P], BF16, tag="wkq")
                                nc.sync.dma_start(wt[:], wq_d[ts(dc, P), ts(mo, P)])
                                for qb in range(n_qb):
                                    nc.tensor.matmul(
                                        pss[qb][:], wt[:], XNQ[:, dc, ts(qb, QB)],
                                        start=(dc == 0), stop=(dc == n_dc - 1))
                            for qb in range(n_qb):
                                nc.vector.tensor_scalar_add(QT[:, mo, ts(qb, QB)],
                                                            pss[qb][:],
                                                            bq_t[:, mo:mo + 1])
                        # V: lhsT = XN chunk (tokens as M), rhs = Wv streamed
                        # per token-group (re-read n_tk/TG times)
                        NO = min(TB, Dm)
                        n_no = Dm // NO
                        TG = 4
                        for tg in range(0, n_tk, TG):
                            tos = range(tg, min(tg + TG, n_tk))
                            for no in range(n_no):
                                pss = [ps_mm.tile([P, NO], F32, tag="ps_mm",
                                                  name="ps_mm") for _ in tos]
                                for dc in range(n_dc):
                                    wvt = p_str.tile([P, NO], BF16, tag="wv")
                                    nc.sync.dma_start(
                                        wvt[:], wv_d[ts(dc, P), ts(no, NO)])
                                    for i, to in enumerate(tos):
                                        nc.tensor.matmul(
                                            pss[i][:], XN[:, dc, ts(to, P)],
                                            wvt[:],
                                            start=(dc == 0), stop=(dc == n_dc - 1))
                                for i, to in enumerate(tos):
                                    nc.vector.tensor_add(VT[:, to, ts(no, NO)],
                                                         pss[i][:],
                                                         bv_bc[:, ts(no, NO)])

                    if dbg:
                        for dc in range(n_dc):
                            nc.sync.dma_start(dbg_d["dKT"][ts(dc, P), :], KT[:, dc, :])
                            nc.sync.dma_start(dbg_d["dQT"][ts(dc, P), :], QT[:, dc, :])
                        for to in range(n_tk):
                            nc.sync.dma_start(dbg_d["dVT"][ts(to, P), :], VT[:, to, :])
                    # ---------- Phase 2: attention ----------
                    with tc.tile_pool(name="p_attn", bufs=1) as p_attn:
                        CT = p_attn.tile([P, n_dc, Q], BF16)
                        for dc in range(n_dc):
                            nc.sync.dma_start(XQ[:, dc, :], xqT_d[ts(dc, P), :])
                        with tc.tile_pool(name="p_exp", bufs=3) as p_exp, \
                             tc.tile_pool(name="p_rb", bufs=3) as p_rb, \
                             tc.tile_pool(name="ps_sc", bufs=4, space="PSUM") as ps_sc, \
                             tc.tile_pool(name="ps_ctx", bufs=2, space="PSUM") as ps_ctx, \
                             tc.tile_pool(name="ps_dn", bufs=2, space="PSUM") as ps_dn, \
                             tc.tile_pool(name="ps_rb", bufs=2, space="PSUM") as ps_rb:
                            for qq in range(n_qq):
                                qsl = ts(qq, QQ)
                                for hp in range(n_hp):
                                    exps = [p_exp.tile([P, n_tk, QQ], BF16,
                                                       tag="expT", name="expT")
                                            for _ in range(2)]
                                    # interleave the two heads' score matmuls:
                                    # they hit different PE row-strips (0/64)
                                    # and run concurrently in the array
                                    for kc in range(n_tk):
                                        for hi in range(2):
                                            r0 = hi * 64
                                            ps_s = ps_sc.tile([P, QQ], F32,
                                                              tag="ps_s")
                                            nc.tensor.matmul(
                                                ps_s[:],
                                                KT[r0:r0 + 64, hp, ts(kc, P)],
                                                QT[r0:r0 + 64, hp, qsl],
                                                start=True, stop=True)
                                            nc.scalar.activation(
                                                exps[hi][:, kc, :], ps_s[:],
                                                AF.Exp, scale=0.125)
                                    rbcs = []
                                    dns = [ps_dn.tile([P, QQ], F32, tag="ps_d",
                                                      name="ps_d")
                                           for _ in range(2)]
                                    for kc in range(n_tk):
                                        for hi in range(2):
                                            nc.tensor.matmul(
                                                dns[hi][:], ones_f[:],
                                                exps[hi][:, kc, :],
                                                start=(kc == 0),
                                                stop=(kc == n_tk - 1))
                                    for hi in range(2):
                                        rbc_h = p_rb.tile([P, QQ], F32, tag="rbc",
                                                          name="rbc")
                                        nc.vector.reciprocal(rbc_h[:], dns[hi][:])
                                        rbcs.append(rbc_h)
                                    if dbg and hp == 0:
                                        nc.sync.dma_start(dbg_d["drbc"][0:64, qsl], rbcs[0][0:64, :])
                                        nc.sync.dma_start(dbg_d["drbc"][64:128, qsl], rbcs[1][64:128, :])
                                        for kc in range(n_tk):
                                            nc.sync.dma_start(
                                                dbg_d["dexp"][ts(kc, P), qsl],
                                                exps[0][:, kc, :])
                                    # interleaved ctx matmuls hit different PE
                                    # col-strips (0/64) -> concurrent
                                    ps_c = ps_ctx.tile([P, QQ], F32, tag="ps_c")
                                    for kc in range(n_tk):
                                        for hi in range(2):
                                            h = 2 * hp + hi
                                            nc.tensor.matmul(
                                                ps_c[hi * 64:hi * 64 + 64, :],
                                                VT[:, kc, ts(h, 64)],
                                                exps[hi][:, kc, :],
                                                start=(kc == 0),
                                                stop=(kc == n_tk - 1),
                                                tile_position=(0, hi * 64))
                                    for hi in range(2):
                                        r0 = hi * 64
                                        nc.vector.tensor_mul(
                                            CT[r0:r0 + 64, hp, qsl],
                                            ps_c[r0:r0 + 64, :],
                                            rbcs[hi][r0:r0 + 64, :])

                        if dbg:
                            for dc in range(n_dc):
                                nc.sync.dma_start(dbg_d["dCT"][ts(dc, P), :], CT[:, dc, :])
                        # Wo + bias + residual into XQ
                        with tc.tile_pool(name="p_wos", bufs=4) as p_wos, \
                             tc.tile_pool(name="ps_wo", bufs=4, space="PSUM") as ps_wo:
                            for mo in range(n_dc):
                                pss = [ps_wo.tile([P, QB], F32, tag="ps_wo", name="ps_wo")
                                       for _ in range(n_qb)]
                                for dc in range(n_dc):
                                    wt = p_wos.tile([P, P], BF16, tag="wo")
                                    nc.sync.dma_start(wt[:],
                                                      wo_d[ts(dc, P), ts(mo, P)])
                                    for qb in range(n_qb):
                                        nc.tensor.matmul(
                                            pss[qb][:], wt[:], CT[:, dc, ts(qb, QB)],
                                            start=(dc == 0), stop=(dc == n_dc - 1))
                                for qb in range(n_qb):
                                    nc.vector.tensor_add(XQ[:, mo, ts(qb, QB)],
                                                         pss[qb][:],
                                                         XQ[:, mo, ts(qb, QB)])
                                    nc.vector.tensor_scalar_add(
                                        XQ[:, mo, ts(qb, QB)],
                                        XQ[:, mo, ts(qb, QB)],
                                        bo_t[:, mo:mo + 1])

                if dbg:
                    for dc in range(n_dc):
                        nc.sync.dma_start(dbg_d["dh2"][ts(dc, P), :], XQ[:, dc, :])
                # ---------- Phase 3: LN2 + MLP ----------
                with tc.tile_pool(name="p_mlp", bufs=1) as p_mlp, \
                     tc.tile_pool(name="p_w1", bufs=3) as p_w1, \
                     tc.tile_pool(name="p_w2", bufs=3) as p_w2, \
                     tc.tile_pool(name="p_tmp2", bufs=2) as p_tmp2, \
                     tc.tile_pool(name="p_st2", bufs=1) as p_st2, \
                     tc.tile_pool(name="p_out", bufs=3) as p_out, \
                     tc.tile_pool(name="ps_st2", bufs=2, space="PSUM") as ps_st2, \
                     tc.tile_pool(name="ps_f", bufs=4, space="PSUM") as ps_f:

                    XN2 = p_mlp.tile([P, n_dc, Q], F32R)
                    _layernorm(nc, ones_h, eps_t, p_mlp, p_tmp2, p_st2, ps_st2,
                               lambda dc: XQ[:, dc, :], n_dc, Q, QB,
                               g2_t, be2_t, lambda dc: XN2[:, dc, :])

                    # weight-outer loops so W1/W2 are read once; Y1 bf16 full-Q
                    Y1 = p_mlp.tile([P, n_mo, Q], BF16, tag="y1")
                    for mo in range(n_mo):
                        wt = p_w1.tile([P, n_dc, P], F32R, tag="w1")
                        nc.sync.dma_start(
                            wt[:],
                            w1_d[:, ts(mo, P)].rearrange("(c p) m -> p c m", p=P))
                        for qb in range(n_qb):
                            ps = ps_f.tile([P, QB], F32, tag="ps_f")
                            for dc in range(n_dc):
                                nc.tensor.matmul(ps[:], wt[:, dc, :],
                                                 XN2[:, dc, ts(qb, QB)],
                                                 start=(dc == 0),
                                                 stop=(dc == n_dc - 1))
                            nc.scalar.activation(Y1[:, mo, ts(qb, QB)], ps[:],
                                                 AF.Gelu, bias=b1_t[:, mo:mo + 1])
                    n_mh = max(1, n_mo // 2)
                    for mo2 in range(n_dc):
                        w2ts = []
                        for half in range(n_mo // n_mh):
                            wt = p_w2.tile([P, n_mh, P], BF16, tag="w2", name="w2")
                            nc.sync.dma_start(
                                wt[:],
                                w2_d[ts(half, n_mh * P), ts(mo2, P)]
                                .rearrange("(c p) m -> p c m", p=P))
                            w2ts.append(wt)
                        for qb in range(n_qb):
                            qsl = ts(qb, QB)
                            ps = ps_f.tile([P, QB], F32, tag="ps_f")
                            for kc in range(n_mo):
                                nc.tensor.matmul(ps[:],
                                                 w2ts[kc // n_mh][:, kc % n_mh, :],
                                                 Y1[:, kc, qsl],
                                                 start=(kc == 0),
                                                 stop=(kc == n_mo - 1))
                            ot = p_out.tile([P, QB], F32, tag="out")
                            nc.vector.tensor_add(ot[:], ps[:], XQ[:, mo2, qsl])
                            nc.vector.tensor_scalar_add(ot[:], ot[:],
                                                        b2_t[:, mo2:mo2 + 1])
                            nc.sync.dma_start(yT_d[ts(mo2, P), qsl], ot[:])
    nc.compile()
    return nc


_NC_CACHE = {}


def _get_nc(T, Q, Dm, Hh, Mlp, n_cores):
    key = (T, Q, Dm, Hh, Mlp, n_cores)
    if key not in _NC_CACHE:
        _NC_CACHE[key] = build_bass(T, Q, Dm, Hh, Mlp, n_cores)
    return _NC_CACHE[key]


def make_in_maps(inputs, n_cores):
    """Per-core input dicts for the (batch x seq-half) sharding."""
    x = np.asarray(inputs["x"], np.float32)
    Bq, Sq, Dq = x.shape
    Qtok = Sq * Bq // n_cores
    bf = ml_dtypes.bfloat16
    shared = {
        "g1": np.asarray(inputs["ln1_g"], np.float32),
        "be1": np.asarray(inputs["ln1_b"], np.float32),
        "g2": np.asarray(inputs["ln2_g"], np.float32),
        "be2": np.asarray(inputs["ln2_b"], np.float32),
        "wq16": np.asarray(inputs["Wq"], np.float32).astype(bf),
        "wk16": np.asarray(inputs["Wk"], np.float32).astype(bf),
        "wv16": np.asarray(inputs["Wv"], np.float32).astype(bf),
        "wo16": np.asarray(inputs["Wo"], np.float32).astype(bf),
        "w1r": np.asarray(inputs["W1"], np.float32),
        "w2r16": np.asarray(inputs["W2"], np.float32).astype(bf),
        "bq": np.asarray(inputs["bq"], np.float32),
        "bk": np.asarray(inputs["bk"], np.float32),
        "bv": np.asarray(inputs["bv"], np.float32),
        "bo": np.asarray(inputs["bo"], np.float32),
        "b1": np.asarray(inputs["b1"], np.float32),
        "b2": np.asarray(inputs["b2"], np.float32),
        "ones16": np.ones((P, 1), bf),
    }
    in_maps = []
    for c in range(n_cores):
        b = c // (n_cores // Bq)
        qoff = (c % (n_cores // Bq)) * Qtok
        m = dict(shared)
        m["xT"] = np.ascontiguousarray(x[b].T)
        m["xqT"] = np.ascontiguousarray(x[b, qoff:qoff + Qtok].T)
        in_maps.append(m)
    return in_maps, Qtok


def kernel(**inputs):
    x = np.asarray(inputs["x"], np.float32)
    Bq, Sq, Dq = x.shape
    in_maps, Qtok = make_in_maps(inputs, N_CORES)
    nc = _get_nc(Sq, Qtok, Dq, H, MLP, N_CORES)
    res = run_bass_kernel_spmd(nc, in_maps, core_ids=list(range(N_CORES)))
    out = np.empty((Bq, Sq, Dq), np.float32)
    per_b = N_CORES // Bq
    for c in range(N_CORES):
        b = c // per_b
        qoff = (c % per_b) * Qtok
        out[b, qoff:qoff + Qtok, :] = res.results[c]["yT"].T
    return out
